# revision 1
# baseline (speedup 1.0000x reference)
"""GAT kernel for trn2, n-core SPMD. Development version."""
import numpy as np
import ml_dtypes

import concourse.bass as bass
import concourse.bacc as bacc
import concourse.mybir as mybir
import concourse.tile as tile
from concourse.masks import make_identity

dt = mybir.dt
F32 = dt.float32
BF16 = dt.bfloat16
I16 = dt.int16
I32 = dt.int32

SENT_NEG = -1.0e30
J_W = 16           # chunks per window
W_E = J_W * 128    # edges per window


class GatConfig:
    def __init__(self, n_nodes=20000, n_graphs=128, n_cores=8,
                 d_in=256, h1=8, d1=64, d2=128):
        self.n_nodes = n_nodes
        self.n_graphs = n_graphs
        self.n_cores = n_cores
        self.d_in = d_in
        self.h1 = h1
        self.d1 = d1
        self.hd1 = h1 * d1        # 512
        self.d2 = d2              # 128
        assert n_nodes % n_cores == 0
        self.slice = n_nodes // n_cores
        self.slice_pad = (self.slice + 1 + 127) // 128 * 128
        self.trows = n_cores * self.slice_pad
        self.n_tiles = self.slice_pad // 128
        r1 = self.hd1 * 2 + 2 * h1 * 4
        self.row1b = (r1 + 255) // 256 * 256     # 1280
        self.row1_bf = self.row1b // 2
        self.row1_f32 = self.row1b // 4
        self.el1_f32 = self.hd1 // 2             # f32 col where el starts
        r2 = d2 * 2 + 2 * 4
        self.row2b = (r2 + 255) // 256 * 256     # 512
        self.row2_bf = self.row2b // 2
        self.row2_f32 = self.row2b // 4
        self.el2_f32 = d2 // 2
        self.sent_row = self.slice
        assert self.row1_f32 - self.el1_f32 >= 64
        assert self.row2_f32 - self.el2_f32 >= 64


def build_host_data(cfg, x, W1, a_l1, a_r1, b1, W2, a_l2, a_r2, b2,
                    edge_src, edge_dst, graph_ids):
    c = cfg
    x = np.asarray(x, np.float32)
    W1 = np.asarray(W1, np.float32); W2 = np.asarray(W2, np.float32)
    a_l1 = np.asarray(a_l1, np.float32); a_r1 = np.asarray(a_r1, np.float32)
    a_l2 = np.asarray(a_l2, np.float32); a_r2 = np.asarray(a_r2, np.float32)
    src = np.asarray(edge_src).astype(np.int64)
    dst = np.asarray(edge_dst).astype(np.int64)
    gid = np.asarray(graph_ids).astype(np.int64)

    w_el1 = np.stack([W1[:, h * c.d1:(h + 1) * c.d1] @ a_l1[h] for h in range(c.h1)], 1)
    w_er1 = np.stack([W1[:, h * c.d1:(h + 1) * c.d1] @ a_r1[h] for h in range(c.h1)], 1)
    W1p = np.concatenate([W1, w_el1, w_er1], axis=1)
    W2p = np.concatenate([W2, W2 @ a_l2[0][:, None], W2 @ a_r2[0][:, None]], axis=1)

    perm = np.argsort(dst, kind="stable")
    src_s = src[perm]; dst_s = dst[perm]
    deg = np.bincount(dst_s, minlength=c.n_nodes)

    core_windows = []
    n_win = 0
    for cc in range(c.n_cores):
        nlo, nhi = cc * c.slice, (cc + 1) * c.slice
        wins = []
        n0 = nlo
        while n0 < nhi:
            n1 = n0
            ecount = 0
            while n1 < nhi and (n1 - n0) < 128 and ecount + deg[n1] <= W_E:
                ecount += deg[n1]
                n1 += 1
            assert n1 > n0, f"node {n0} degree {deg[n0]} > {W_E}"
            wins.append((n0, n1))
            n0 = n1
        core_windows.append(wins)
        n_win = max(n_win, len(wins))
    epad = n_win * W_E
    jtot = epad // 128

    edge_starts = np.searchsorted(dst_s, np.arange(c.n_nodes + 1))

    per_core = []
    for cc in range(c.n_cores):
        wins = core_windows[cc]
        hrows = c.slice_pad // 2
        sent_g = ((c.sent_row // hrows) * c.n_cores * hrows + 0 * hrows
                  + c.sent_row % hrows)
        src_g = np.full(epad, sent_g, np.int64)
        dst_g = np.full(epad, sent_g, np.int64)
        dstrel = np.zeros(epad, np.float32)
        hidx = np.full((128, n_win), c.slice_pad, np.int64)
        gidw = np.full((128, n_win), 999.0, np.float32)
        for w, (n0, n1) in enumerate(wins):
            e0, e1 = edge_starts[n0], edge_starts[n1]
            ne = e1 - e0
            base = w * W_E
            es = src_s[e0:e1]; ed = dst_s[e0:e1]
            ec, er_ = es // c.slice, es % c.slice
            dc, dr_ = ed // c.slice, ed % c.slice
            hrows = c.slice_pad // 2
            src_g[base:base + ne] = ((er_ // hrows) * c.n_cores * hrows
                                     + ec * hrows + er_ % hrows)
            dst_g[base:base + ne] = ((dr_ // hrows) * c.n_cores * hrows
                                     + dc * hrows + dr_ % hrows)
            dstrel[base:base + ne] = (ed - n0).astype(np.float32)
            nw = n1 - n0
            hidx[:nw, w] = (n0 - cc * c.slice) + np.arange(nw)
            gidw[:nw, w] = gid[n0:n1].astype(np.float32)
        assert src_g.max() < 32768 and dst_g.max() < 32768
        src_i16 = np.tile(src_g.astype(np.int16).reshape(epad // 16, 16).T, (8, 1)).copy()
        dst_i16 = np.tile(dst_g.astype(np.int16).reshape(epad // 16, 16).T, (8, 1)).copy()
        dstrel_t = dstrel.reshape(jtot, 128).T.copy()

        nlo = cc * c.slice
        xT = np.zeros((c.d_in, c.slice_pad), np.float32)
        xT[:, :c.slice] = x[nlo:nlo + c.slice].T

        per_core.append({
            "xT": xT, "w1p": W1p, "w2p": W2p,
            "b1t": np.tile(np.asarray(b1, np.float32)[None, :], (128, 1)),
            "b2t": np.tile(np.asarray(b2, np.float32)[None, :], (128, 1)),
            "srcidx": src_i16, "dstidx": dst_i16, "dstrel": dstrel_t,
            "hidx": np.ascontiguousarray(hidx.astype(np.int32)),
            "gidw": np.ascontiguousarray(gidw),
            "iota": np.tile(np.arange(128, dtype=np.float32).astype(ml_dtypes.bfloat16),
                            (128, 1)).copy(),
        })

    counts = np.bincount(gid, minlength=c.n_graphs).astype(np.float32)
    has_b = (bool(np.any(np.asarray(b1))), bool(np.any(np.asarray(b2))))
    return per_core, counts, n_win, has_b



def _gather2(nc, out_tile, in_ap, idx, wcol, n, elem, step=None):
    """Two 1024-idx dma_gathers (SWDGE ring caps one instruction at 128 descs)."""
    half = n // 2
    for g in range(2):
        nc.gpsimd.dma_gather(
            out_ap=out_tile[:, g * (half // 128):(g + 1) * (half // 128)],
            in_ap=in_ap,
            idxs_ap=idx[:, wcol + g * (half // 16):wcol + (g + 1) * (half // 16)],
            num_idxs=half, num_idxs_reg=half, elem_size=elem,
            **({"elem_step": step} if step is not None else {}))

def build_program(cfg, n_win, sim1=False, fake_ag=False, has_b=(False, False)):
    fake_ag = fake_ag or sim1
    c = cfg
    epad = n_win * W_E
    jtot = epad // 128
    nc = bacc.Bacc("TRN2", target_bir_lowering=False, debug=False,
                   num_devices=1 if sim1 else c.n_cores)

    t_xT = nc.dram_tensor("xT", [c.d_in, c.slice_pad], F32, kind="ExternalInput")
    t_w1p = nc.dram_tensor("w1p", [c.d_in, c.hd1 + 2 * c.h1], F32, kind="ExternalInput")
    t_w2p = nc.dram_tensor("w2p", [c.hd1, c.d2 + 2], F32, kind="ExternalInput")
    t_srcidx = nc.dram_tensor("srcidx", [128, epad // 16], I16, kind="ExternalInput")
    t_dstidx = nc.dram_tensor("dstidx", [128, epad // 16], I16, kind="ExternalInput")
    t_dstrel = nc.dram_tensor("dstrel", [128, jtot], F32, kind="ExternalInput")
    t_hidx = nc.dram_tensor("hidx", [128, n_win], I32, kind="ExternalInput")
    t_gidw = nc.dram_tensor("gidw", [128, n_win], F32, kind="ExternalInput")
    t_iota = nc.dram_tensor("iota", [128, 128], BF16, kind="ExternalInput")
    t_b1t = nc.dram_tensor("b1t", [128, c.hd1], F32, kind="ExternalInput")
    t_b2t = nc.dram_tensor("b2t", [128, c.d2], F32, kind="ExternalInput")
    t_pool = nc.dram_tensor("pool", [128, c.d2], F32, kind="ExternalOutput")

    bounce1 = nc.dram_tensor("bounce1", [c.slice_pad, c.row1_bf], BF16)
    table1 = nc.dram_tensor("table1", [c.trows, c.row1_bf], BF16)
    bounce2 = nc.dram_tensor("bounce2", [c.slice_pad, c.row2_bf], BF16)
    table2 = nc.dram_tensor("table2", [c.trows, c.row2_bf], BF16)
    htab = nc.dram_tensor("htab", [c.slice_pad + 8, c.hd1], F32)

    K1 = c.d_in // 128
    K2 = c.hd1 // 128
    NW1 = c.hd1 + 2 * c.h1
    NW2 = c.d2 + 2
    h1, d1, hd1, d2 = c.h1, c.d1, c.hd1, c.d2

    with tile.TileContext(nc) as tc:
        with (
            tc.tile_pool(name="res", bufs=1) as res,
            tc.tile_pool(name="wk", bufs=3) as wk,
            tc.tile_pool(name="gat", bufs=2) as gat,
            tc.tile_pool(name="chunk", bufs=6) as chk,
            tc.tile_pool(name="ps", bufs=4, space="PSUM") as ps,
            tc.tile_pool(name="pss", bufs=2, space="PSUM") as pss,
            tc.tile_pool(name="pspool", bufs=1, space="PSUM") as pspool,
        ):
            srcidx = res.tile([128, epad // 16], I16)
            dstidx = res.tile([128, epad // 16], I16)
            dstrel = res.tile([128, jtot], F32)
            hidx = res.tile([128, n_win], I32)
            gidw = res.tile([128, n_win], F32)
            iota = res.tile([128, 128], BF16)
            nc.sync.dma_start(out=srcidx[:], in_=t_srcidx[:])
            nc.sync.dma_start(out=dstidx[:], in_=t_dstidx[:])
            nc.sync.dma_start(out=dstrel[:], in_=t_dstrel[:])
            nc.sync.dma_start(out=hidx[:], in_=t_hidx[:])
            nc.sync.dma_start(out=gidw[:], in_=t_gidw[:])
            nc.sync.dma_start(out=iota[:], in_=t_iota[:])
            b1t = res.tile([128, hd1], F32)
            b2t = res.tile([128, d2], F32)
            if has_b[0]:
                nc.sync.dma_start(out=b1t[:], in_=t_b1t[:])
            if has_b[1]:
                nc.sync.dma_start(out=b2t[:], in_=t_b2t[:])
            ident = res.tile([128, 128], F32)
            make_identity(nc, ident[:])
            zeros = res.tile([128, hd1], F32)
            nc.vector.memset(zeros[:], 0.0)

            w1k = []
            for k in range(K1):
                t = res.tile([128, NW1], F32, tag=f"w1k{k}")
                nc.sync.dma_start(out=t[:], in_=t_w1p[k * 128:(k + 1) * 128, :])
                w1k.append(t)
            w2k = []
            for k in range(K2):
                t = res.tile([128, NW2], F32, tag=f"w2k{k}")
                nc.sync.dma_start(out=t[:], in_=t_w2p[k * 128:(k + 1) * 128, :])
                w2k.append(t)

            b1f32 = bounce1[:].bitcast(F32)
            t1f32 = table1[:].bitcast(F32)
            b2f32 = bounce2[:].bitcast(F32)
            t2f32 = table2[:].bitcast(F32)

            # ---- Phase A ----
            for t in range(c.n_tiles):
                xts = []
                for k in range(K1):
                    xt = wk.tile([128, 128], F32, tag=f"xt{k}")
                    nc.sync.dma_start(out=xt[:], in_=t_xT[k * 128:(k + 1) * 128,
                                                         t * 128:(t + 1) * 128])
                    xts.append(xt)
                psf = ps.tile([128, hd1], F32, space="PSUM", tag="big")
                pse = pss.tile([128, 2 * h1], F32, space="PSUM", tag="small")
                for k in range(K1):
                    nc.tensor.matmul(out=psf[:], lhsT=xts[k][:], rhs=w1k[k][:, 0:hd1],
                                     start=(k == 0), stop=(k == K1 - 1))
                for k in range(K1):
                    nc.tensor.matmul(out=pse[:], lhsT=xts[k][:], rhs=w1k[k][:, hd1:NW1],
                                     start=(k == 0), stop=(k == K1 - 1))
                fbf = wk.tile([128, hd1], BF16, tag="fbf")
                nc.vector.tensor_copy(out=fbf[:], in_=psf[:])
                elr = wk.tile([128, 2 * h1], F32, tag="elr")
                nc.vector.tensor_copy(out=elr[:], in_=pse[:])
                r0, r1 = t * 128, (t + 1) * 128
                nc.sync.dma_start(out=bounce1[r0:r1, 0:hd1], in_=fbf[:])
                nc.sync.dma_start(out=b1f32[r0:r1, c.el1_f32:c.el1_f32 + 2 * h1],
                                  in_=elr[:])
            sent = wk.tile([1, 2 * h1], F32, tag="sent")
            nc.vector.memset(sent[:], SENT_NEG)
            nc.sync.dma_start(
                out=b1f32[c.sent_row:c.sent_row + 1, c.el1_f32:c.el1_f32 + 2 * h1],
                in_=sent[:])

            hrows = c.slice_pad // 2
            gh = c.n_cores * hrows
            for hh in range(2):
                r0, r1 = hh * hrows, (hh + 1) * hrows
                if fake_ag:
                    for cc in range(c.n_cores):
                        nc.sync.dma_start(
                            out=table1[hh * gh + cc * hrows:hh * gh + (cc + 1) * hrows, :],
                            in_=bounce1[r0:r1, :])
                else:
                    nc.gpsimd.collective_compute(
                        "AllGather", mybir.AluOpType.bypass,
                        replica_groups=[list(range(c.n_cores))],
                        ins=[bounce1[r0:r1, :]], outs=[table1[hh * gh:(hh + 1) * gh, :]])

            if c.slice_pad > c.slice:
                npad = c.slice_pad - c.slice
                nc.sync.dma_start(out=htab[c.slice:c.slice_pad, :],
                                  in_=zeros[:npad, :])

            # ---- Phase B: layer-1 edges ----
            for w in range(n_win):
                featg = gat.tile([128, J_W, c.row1_bf], BF16, tag="featg")
                _gather2(nc, featg, table1[:], srcidx, w * 128, W_E, c.row1_bf)
                erg = gat.tile([128, J_W, 64], F32, tag="erg")
                _gather2(nc, erg, t1f32[:, c.el1_f32:c.el1_f32 + 64], dstidx,
                         w * 128, W_E, 64, step=c.row1_f32)
                fg32 = featg[:].bitcast(F32)
                elv = fg32[:, :, c.el1_f32:c.el1_f32 + h1]
                erv = erg[:, :, h1:2 * h1]
                s = wk.tile([128, J_W, h1], F32, tag="s1")
                nc.vector.tensor_tensor(out=s[:], in0=elv, in1=erv,
                                        op=mybir.AluOpType.add)
                slr = wk.tile([128, J_W, h1], F32, tag="slr1")
                nc.vector.tensor_scalar_mul(slr[:], s[:], 0.2)
                slr2 = wk.tile([128, J_W, h1], F32, tag="slr1b")
                nc.vector.tensor_tensor(out=slr2[:], in0=slr[:], in1=s[:],
                                        op=mybir.AluOpType.max)
                pexp = gat.tile([128, J_W, h1, d1], BF16, tag="pexp")
                dre = gat.tile([128, J_W, 128], BF16, tag="dre")
                GRP = 4
                for g in range(0, J_W, GRP):
                    nc.scalar.activation(
                        out=dre[:, g:g + GRP],
                        in_=dstrel[:, w * J_W + g:w * J_W + g + GRP]
                            .to_broadcast([128, GRP, 128]),
                        func=mybir.ActivationFunctionType.Copy)
                    nc.scalar.activation(
                        out=pexp[:, g:g + GRP],
                        in_=slr2[:, g:g + GRP].to_broadcast([128, GRP, h1, d1]),
                        func=mybir.ActivationFunctionType.Exp)

                psr = ps.tile([128, hd1], F32, space="PSUM", tag="big")
                psd = pss.tile([128, h1], F32, space="PSUM", tag="small")
                for ci in range(J_W):
                    ind = chk.tile([128, 128], BF16, tag="ind")
                    nc.vector.tensor_tensor(out=ind[:], in0=dre[:, ci],
                                            in1=iota[:], op=mybir.AluOpType.is_equal)
                    msg = chk.tile([128, hd1], BF16, tag="msg")
                    nc.vector.tensor_tensor(out=msg[:], in0=featg[:, ci, 0:hd1],
                                            in1=pexp[:, ci],
                                            op=mybir.AluOpType.mult)
                    nc.tensor.matmul(out=psr[:], lhsT=ind[:], rhs=msg[:],
                                     start=(ci == 0), stop=(ci == J_W - 1))
                    nc.tensor.matmul(out=psd[:], lhsT=ind[:], rhs=pexp[:, ci, :, 0],
                                     start=(ci == 0), stop=(ci == J_W - 1))
                dg = wk.tile([128, h1], F32, tag="dg1")
                nc.vector.tensor_scalar_max(dg[:], psd[:], 1e-30)
                rec = wk.tile([128, h1], F32, tag="rec1")
                nc.vector.reciprocal(out=rec[:], in_=dg[:])
                hwin = wk.tile([128, hd1], F32, tag="hwin")
                nc.vector.tensor_tensor(out=hwin[:], in0=psr[:],
                                        in1=rec[:].to_broadcast([128, h1, d1]),
                                        op=mybir.AluOpType.mult)
                if has_b[0]:
                    nc.vector.tensor_tensor(out=hwin[:], in0=hwin[:], in1=b1t[:],
                                            op=mybir.AluOpType.add)
                if sim1:
                    nc.gpsimd.dma_start(
                        out=htab[(w % c.n_tiles) * 128:(w % c.n_tiles) * 128 + 128, :],
                        in_=hwin[:])
                else:
                    nc.gpsimd.indirect_dma_start(
                        out=htab[:],
                        out_offset=bass.IndirectOffsetOnAxis(ap=hidx[:, w:w + 1], axis=0),
                        in_=hwin[:], in_offset=None)

            # ---- Phase C: feat2 ----
            for t in range(c.n_tiles):
                ht = wk.tile([128, hd1], F32, tag="ht")
                nc.sync.dma_start(out=ht[:], in_=htab[t * 128:(t + 1) * 128, :])
                hTs = []
                for k in range(K2):
                    pst = ps.tile([128, 128], F32, space="PSUM", tag="big")
                    nc.tensor.transpose(out=pst[:], in_=ht[:, k * 128:(k + 1) * 128],
                                        identity=ident[:])
                    hT = wk.tile([128, 128], F32, tag=f"hT{k}")
                    nc.vector.tensor_copy(out=hT[:], in_=pst[:])
                    hTs.append(hT)
                psf2 = ps.tile([128, NW2], F32, space="PSUM", tag="big")
                for k in range(K2):
                    nc.tensor.matmul(out=psf2[:], lhsT=hTs[k][:], rhs=w2k[k][:],
                                     start=(k == 0), stop=(k == K2 - 1))
                f2bf = wk.tile([128, d2], BF16, tag="f2bf")
                nc.vector.tensor_copy(out=f2bf[:], in_=psf2[:, 0:d2])
                elr2 = wk.tile([128, 2], F32, tag="elr2")
                nc.vector.tensor_copy(out=elr2[:], in_=psf2[:, d2:d2 + 2])
                r0, r1 = t * 128, (t + 1) * 128
                nc.sync.dma_start(out=bounce2[r0:r1, 0:d2], in_=f2bf[:])
                nc.sync.dma_start(out=b2f32[r0:r1, c.el2_f32:c.el2_f32 + 2],
                                  in_=elr2[:])
            sent2 = wk.tile([1, 2], F32, tag="sent2")
            nc.vector.memset(sent2[:], SENT_NEG)
            nc.sync.dma_start(
                out=b2f32[c.sent_row:c.sent_row + 1, c.el2_f32:c.el2_f32 + 2],
                in_=sent2[:])

            for hh in range(2):
                r0, r1 = hh * hrows, (hh + 1) * hrows
                if fake_ag:
                    for cc in range(c.n_cores):
                        nc.sync.dma_start(
                            out=table2[hh * gh + cc * hrows:hh * gh + (cc + 1) * hrows, :],
                            in_=bounce2[r0:r1, :])
                else:
                    nc.gpsimd.collective_compute(
                        "AllGather", mybir.AluOpType.bypass,
                        replica_groups=[list(range(c.n_cores))],
                        ins=[bounce2[r0:r1, :]], outs=[table2[hh * gh:(hh + 1) * gh, :]])

            # ---- Phase D: layer-2 edges + pooling ----
            pspl = pspool.tile([128, d2], F32, space="PSUM", tag="pspl")
            for w in range(n_win):
                f2g = gat.tile([128, J_W, c.row2_bf], BF16, tag="f2g")
                _gather2(nc, f2g, table2[:], srcidx, w * 128, W_E, c.row2_bf)
                er2g = gat.tile([128, J_W, 64], F32, tag="er2g")
                _gather2(nc, er2g, t2f32[:, c.el2_f32:c.el2_f32 + 64], dstidx,
                         w * 128, W_E, 64, step=c.row2_f32)
                f2g32 = f2g[:].bitcast(F32)
                el2v = f2g32[:, :, c.el2_f32:c.el2_f32 + 1]
                er2v = er2g[:, :, 1:2]
                s = wk.tile([128, J_W, 1], F32, tag="s2")
                nc.vector.tensor_tensor(out=s[:], in0=el2v, in1=er2v,
                                        op=mybir.AluOpType.add)
                slr = wk.tile([128, J_W, 1], F32, tag="slr2_")
                nc.vector.tensor_scalar_mul(slr[:], s[:], 0.2)
                slr2 = wk.tile([128, J_W, 1], F32, tag="slr2b")
                nc.vector.tensor_tensor(out=slr2[:], in0=slr[:], in1=s[:],
                                        op=mybir.AluOpType.max)
                pexp2 = gat.tile([128, J_W, 1, d2], BF16, tag="pexp2")
                dre = gat.tile([128, J_W, 128], BF16, tag="dre2")
                GRP = 4
                for g in range(0, J_W, GRP):
                    nc.scalar.activation(
                        out=dre[:, g:g + GRP],
                        in_=dstrel[:, w * J_W + g:w * J_W + g + GRP]
                            .to_broadcast([128, GRP, 128]),
                        func=mybir.ActivationFunctionType.Copy)
                    nc.scalar.activation(
                        out=pexp2[:, g:g + GRP],
                        in_=slr2[:, g:g + GRP].to_broadcast([128, GRP, 1, d2]),
                        func=mybir.ActivationFunctionType.Exp)

                psr2 = ps.tile([128, d2], F32, space="PSUM", tag="big")
                psd2 = pss.tile([128, 8], F32, space="PSUM", tag="small")
                for ci in range(J_W):
                    ind = chk.tile([128, 128], BF16, tag="ind2")
                    nc.vector.tensor_tensor(out=ind[:], in0=dre[:, ci],
                                            in1=iota[:], op=mybir.AluOpType.is_equal)
                    msg = chk.tile([128, d2], BF16, tag="msg2")
                    nc.vector.tensor_tensor(out=msg[:], in0=f2g[:, ci, 0:d2],
                                            in1=pexp2[:, ci, 0],
                                            op=mybir.AluOpType.mult)
                    nc.tensor.matmul(out=psr2[:], lhsT=ind[:], rhs=msg[:],
                                     start=(ci == 0), stop=(ci == J_W - 1))
                    nc.tensor.matmul(out=psd2[:, 0:1], lhsT=ind[:],
                                     rhs=pexp2[:, ci, :, 0],
                                     start=(ci == 0), stop=(ci == J_W - 1))
                dg = wk.tile([128, 1], F32, tag="dg2")
                nc.vector.tensor_scalar_max(dg[:], psd2[:, 0:1], 1e-30)
                rec = wk.tile([128, 1], F32, tag="rec2")
                nc.vector.reciprocal(out=rec[:], in_=dg[:])
                h2w = wk.tile([128, d2], F32, tag="h2w")
                nc.vector.tensor_tensor(out=h2w[:], in0=psr2[:],
                                        in1=rec[:].to_broadcast([128, d2]),
                                        op=mybir.AluOpType.mult)
                if has_b[1]:
                    nc.vector.tensor_tensor(out=h2w[:], in0=h2w[:], in1=b2t[:],
                                            op=mybir.AluOpType.add)
                gexp = wk.tile([128, 128], BF16, tag="gexp")
                nc.scalar.activation(out=gexp[:],
                                     in_=gidw[:, w:w + 1].to_broadcast([128, 128]),
                                     func=mybir.ActivationFunctionType.Copy)
                gind = wk.tile([128, 128], F32, tag="gind")
                nc.vector.tensor_tensor(out=gind[:], in0=gexp[:], in1=iota[:],
                                        op=mybir.AluOpType.is_equal)
                nc.tensor.matmul(out=pspl[:], lhsT=gind[:], rhs=h2w[:],
                                 start=(w == 0), stop=(w == n_win - 1))

            poolsb = res.tile([128, d2], F32)
            nc.vector.tensor_copy(out=poolsb[:], in_=pspl[:])
            nc.sync.dma_start(out=t_pool[:], in_=poolsb[:])

    nc.compile()
    return nc


def host_reduce(cfg, results, counts):
    pool_sum = np.zeros((128, cfg.d2), np.float32)
    for cc in range(cfg.n_cores):
        pool_sum += np.asarray(results[cc]["pool"], np.float32)
    hg = pool_sum[:cfg.n_graphs] / np.maximum(counts, 1.0)[:, None]
    return hg.reshape(cfg.n_graphs, 1, cfg.d2).astype(np.float32)




_PROG_CACHE = {}


def kernel(**inputs):
    """GAT forward on 8 trn2 NeuronCores. Full inputs in, [128,1,128] f32 out."""
    import concourse.bass_utils as bass_utils
    cfg = GatConfig()
    per_core, counts, n_win, has_b = build_host_data(cfg, **inputs)
    key = (n_win, has_b)
    nc = _PROG_CACHE.get(key)
    if nc is None:
        nc = build_program(cfg, n_win, has_b=has_b)
        _PROG_CACHE[key] = nc
    res = bass_utils.run_bass_kernel_spmd(nc, per_core,
                                          core_ids=list(range(cfg.n_cores)))
    return host_reduce(cfg, res.results, counts)



# revision 30
# speedup vs baseline: 2.2802x; 2.2802x over previous
"""GAT kernel for trn2, n-core SPMD. Development version."""
import numpy as np
import ml_dtypes

import concourse.bass as bass
import concourse.bacc as bacc
import concourse.mybir as mybir
import concourse.tile as tile
from concourse.masks import make_identity

dt = mybir.dt
F32 = dt.float32
BF16 = dt.bfloat16
I16 = dt.int16
I32 = dt.int32

SENT_NEG = -1.0e30
J_W = 16           # chunks per window
W_E = J_W * 128    # edges per window


class GatConfig:
    def __init__(self, n_nodes=20000, n_graphs=128, n_cores=8,
                 d_in=256, h1=8, d1=64, d2=128):
        self.n_nodes = n_nodes
        self.n_graphs = n_graphs
        self.n_cores = n_cores
        self.d_in = d_in
        self.h1 = h1
        self.d1 = d1
        self.hd1 = h1 * d1        # 512
        self.d2 = d2              # 128
        assert n_nodes % n_cores == 0
        self.slice = n_nodes // n_cores
        self.slice_pad = (self.slice + 1 + 127) // 128 * 128
        self.trows = n_cores * self.slice_pad
        self.n_tiles = self.slice_pad // 128
        # gather rows/steps must be 256B multiples (SWDGE constraint)
        r1 = self.hd1 * 2 + h1 * 4
        self.row1b = (r1 + 255) // 256 * 256     # 1280
        self.row1_bf = self.row1b // 2
        self.el1_f32 = self.hd1 // 2             # f32 col where el starts
        r2 = d2 * 2 + 4
        self.row2b = (r2 + 255) // 256 * 256     # 512
        self.row2_bf = self.row2b // 2
        self.el2_f32 = d2 // 2
        self.sent_row = self.slice


def build_host_data(cfg, x, W1, a_l1, a_r1, b1, W2, a_l2, a_r2, b2,
                    edge_src, edge_dst, graph_ids):
    c = cfg
    x = np.asarray(x, np.float32)
    W1 = np.asarray(W1, np.float32); W2 = np.asarray(W2, np.float32)
    a_l1 = np.asarray(a_l1, np.float32); a_r1 = np.asarray(a_r1, np.float32)
    a_l2 = np.asarray(a_l2, np.float32); a_r2 = np.asarray(a_r2, np.float32)
    src = np.asarray(edge_src).astype(np.int64)
    dst = np.asarray(edge_dst).astype(np.int64)
    gid = np.asarray(graph_ids).astype(np.int64)

    w_el1 = np.stack([W1[:, h * c.d1:(h + 1) * c.d1] @ a_l1[h] for h in range(c.h1)], 1)
    w_er1 = np.stack([W1[:, h * c.d1:(h + 1) * c.d1] @ a_r1[h] for h in range(c.h1)], 1)
    # d-major head layout: feature col (h, d) -> (d, h) so the per-head
    # attention factor broadcasts over the middle dim (keeps DVE 2x mode)
    W1dm = W1.reshape(c.d_in, c.h1, c.d1).transpose(0, 2, 1).reshape(c.d_in, c.hd1)
    W1p = np.concatenate([W1dm, w_el1, w_er1], axis=1)
    W2dm = W2.reshape(c.h1, c.d1, c.d2).transpose(1, 0, 2).reshape(c.hd1, c.d2)
    W2p = np.concatenate([W2dm, W2dm @ a_l2[0][:, None], W2dm @ a_r2[0][:, None]],
                         axis=1)

    perm = np.argsort(dst, kind="stable")
    src_s = src[perm]; dst_s = dst[perm]
    deg = np.bincount(dst_s, minlength=c.n_nodes)

    core_windows = []
    n_win = 0
    for cc in range(c.n_cores):
        nlo, nhi = cc * c.slice, (cc + 1) * c.slice
        wins = []
        n0 = nlo
        while n0 < nhi:
            n1 = n0
            ecount = 0
            while n1 < nhi and (n1 - n0) < 128 and ecount + deg[n1] <= W_E:
                ecount += deg[n1]
                n1 += 1
            assert n1 > n0, f"node {n0} degree {deg[n0]} > {W_E}"
            wins.append((n0, n1))
            n0 = n1
        core_windows.append(wins)
        n_win = max(n_win, len(wins))
    epad = n_win * W_E
    jtot = epad // 128

    edge_starts = np.searchsorted(dst_s, np.arange(c.n_nodes + 1))

    per_core = []
    for cc in range(c.n_cores):
        wins = core_windows[cc]
        hrows = c.slice_pad // 2
        sent_g = ((c.sent_row // hrows) * c.n_cores * hrows + 0 * hrows
                  + c.sent_row % hrows)
        src_g = np.full(epad, sent_g, np.int64)
        dst_g = np.full(epad, c.sent_row, np.int64)   # local dst rows
        dstrel = np.zeros(epad, np.float32)
        hidx = np.full((128, n_win), c.slice_pad, np.int64)
        gidw = np.full((128, n_win), 999.0, np.float32)
        for w, (n0, n1) in enumerate(wins):
            e0, e1 = edge_starts[n0], edge_starts[n1]
            ne = e1 - e0
            base = w * W_E
            es = src_s[e0:e1]; ed = dst_s[e0:e1]
            ec, er_ = es // c.slice, es % c.slice
            src_g[base:base + ne] = ((er_ // hrows) * c.n_cores * hrows
                                     + ec * hrows + er_ % hrows)
            dst_g[base:base + ne] = ed % c.slice
            dstrel[base:base + ne] = (ed - n0).astype(np.float32)
            nw = n1 - n0
            hidx[:nw, w] = (n0 - cc * c.slice) + np.arange(nw)
            gidw[:nw, w] = gid[n0:n1].astype(np.float32)
        assert src_g.max() < 32768 and dst_g.max() < 32768
        src_i16 = np.tile(src_g.astype(np.int16).reshape(epad // 16, 16).T, (8, 1)).copy()
        dst_i16 = np.tile(dst_g.astype(np.int16).reshape(epad // 16, 16).T, (8, 1)).copy()
        dstrel_t = dstrel.reshape(jtot, 128).T.astype(ml_dtypes.bfloat16).copy()

        nlo = cc * c.slice
        xT = np.zeros((c.d_in, c.slice_pad), np.float32)
        xT[:, :c.slice] = x[nlo:nlo + c.slice].T

        per_core.append({
            "xT": xT, "w1p": W1p, "w2p": W2p,
            "b1t": np.tile(np.asarray(b1, np.float32)
                           .reshape(c.h1, c.d1).T.reshape(-1)[None, :], (128, 1)),
            "b2t": np.tile(np.asarray(b2, np.float32)[None, :], (128, 1)),
            "srcidx": src_i16, "dstidx": dst_i16, "dstrel": dstrel_t,
            "hidx": np.ascontiguousarray(hidx.astype(np.int32)),
            "gidw": np.ascontiguousarray(gidw.astype(ml_dtypes.bfloat16)),
            "iota": np.tile(np.arange(128, dtype=np.float32).astype(ml_dtypes.bfloat16),
                            (128, 1)).copy(),
        })

    counts = np.bincount(gid, minlength=c.n_graphs).astype(np.float32)
    has_b = (bool(np.any(np.asarray(b1))), bool(np.any(np.asarray(b2))))
    return per_core, counts, n_win, has_b



def _gather2(nc, out_tile, in_ap, idx, wcol, n, elem, step=None):
    """Two 1024-idx dma_gathers (SWDGE ring caps one instruction at 128 descs)."""
    half = n // 2
    for g in range(2):
        nc.gpsimd.dma_gather(
            out_ap=out_tile[:, g * (half // 128):(g + 1) * (half // 128)],
            in_ap=in_ap,
            idxs_ap=idx[:, wcol + g * (half // 16):wcol + (g + 1) * (half // 16)],
            num_idxs=half, num_idxs_reg=half, elem_size=elem,
            **({"elem_step": step} if step is not None else {}))

def build_program(cfg, n_win, sim1=False, fake_ag=False, has_b=(False, False)):
    fake_ag = fake_ag or sim1
    c = cfg
    epad = n_win * W_E
    jtot = epad // 128
    nc = bacc.Bacc("TRN2", target_bir_lowering=False, debug=False,
                   num_devices=1 if sim1 else c.n_cores)

    t_xT = nc.dram_tensor("xT", [c.d_in, c.slice_pad], F32, kind="ExternalInput")
    t_w1p = nc.dram_tensor("w1p", [c.d_in, c.hd1 + 2 * c.h1], F32, kind="ExternalInput")
    t_w2p = nc.dram_tensor("w2p", [c.hd1, c.d2 + 2], F32, kind="ExternalInput")
    t_srcidx = nc.dram_tensor("srcidx", [128, epad // 16], I16, kind="ExternalInput")
    t_dstidx = nc.dram_tensor("dstidx", [128, epad // 16], I16, kind="ExternalInput")
    t_dstrel = nc.dram_tensor("dstrel", [128, jtot], BF16, kind="ExternalInput")
    t_hidx = nc.dram_tensor("hidx", [128, n_win], I32, kind="ExternalInput")
    t_gidw = nc.dram_tensor("gidw", [128, n_win], BF16, kind="ExternalInput")
    t_iota = nc.dram_tensor("iota", [128, 128], BF16, kind="ExternalInput")
    t_b1t = nc.dram_tensor("b1t", [128, c.hd1], F32, kind="ExternalInput")
    t_b2t = nc.dram_tensor("b2t", [128, c.d2], F32, kind="ExternalInput")
    t_pool = nc.dram_tensor("pool", [128, c.d2], F32, kind="ExternalOutput")

    shr = {} if sim1 else {"addr_space": "Shared"}
    bounce1 = nc.dram_tensor("bounce1", [c.slice_pad, c.row1_bf], BF16)
    table1 = nc.dram_tensor("table1", [c.trows, c.row1_bf], BF16, **shr)
    bounce2 = nc.dram_tensor("bounce2", [c.slice_pad, c.row2_bf], BF16)
    table2 = nc.dram_tensor("table2", [c.trows, c.row2_bf], BF16, **shr)
    erloc = nc.dram_tensor("erloc", [c.slice_pad, 64], F32)
    er2loc = nc.dram_tensor("er2loc", [c.slice_pad, 64], F32)
    htab = nc.dram_tensor("htab", [c.slice_pad + 8, c.hd1], F32)

    K1 = c.d_in // 128
    K2 = c.hd1 // 128
    NW1 = c.hd1 + 2 * c.h1
    NW2 = c.d2 + 2
    h1, d1, hd1, d2 = c.h1, c.d1, c.hd1, c.d2

    with tile.TileContext(nc) as tc:
        with (
            tc.tile_pool(name="res", bufs=1) as res,
            tc.tile_pool(name="wk", bufs=3) as wk,
            tc.tile_pool(name="gat", bufs=2) as gat,
            tc.tile_pool(name="chunk", bufs=6) as chk,
            tc.tile_pool(name="ps", bufs=4, space="PSUM") as ps,
            tc.tile_pool(name="pss", bufs=2, space="PSUM") as pss,
            tc.tile_pool(name="pspool", bufs=1, space="PSUM") as pspool,
        ):
            srcidx = res.tile([128, epad // 16], I16)
            dstidx = res.tile([128, epad // 16], I16)
            dstrel = res.tile([128, jtot], BF16)
            hidx = res.tile([128, n_win], I32)
            gidw = res.tile([128, n_win], BF16)
            iota = res.tile([128, 128], BF16)
            nc.sync.dma_start(out=srcidx[:], in_=t_srcidx[:])
            nc.sync.dma_start(out=dstidx[:], in_=t_dstidx[:])
            nc.sync.dma_start(out=dstrel[:], in_=t_dstrel[:])
            nc.sync.dma_start(out=hidx[:], in_=t_hidx[:])
            nc.sync.dma_start(out=gidw[:], in_=t_gidw[:])
            nc.sync.dma_start(out=iota[:], in_=t_iota[:])
            b1t = res.tile([128, hd1], F32)
            b2t = res.tile([128, d2], F32)
            if has_b[0]:
                nc.sync.dma_start(out=b1t[:], in_=t_b1t[:])
            if has_b[1]:
                nc.sync.dma_start(out=b2t[:], in_=t_b2t[:])
            ident = res.tile([128, 128], F32)
            make_identity(nc, ident[:])
            zeros = res.tile([128, hd1], F32)
            nc.vector.memset(zeros[:], 0.0)
            ones_bf = res.tile([128, 1], BF16)
            nc.vector.memset(ones_bf[:], 1.0)

            w1k = []
            for k in range(K1):
                t = res.tile([128, NW1], F32, tag=f"w1k{k}")
                nc.sync.dma_start(out=t[:], in_=t_w1p[k * 128:(k + 1) * 128, :])
                w1k.append(t)
            w2k = []
            for k in range(K2):
                t = res.tile([128, NW2], F32, tag=f"w2k{k}")
                nc.sync.dma_start(out=t[:], in_=t_w2p[k * 128:(k + 1) * 128, :])
                w2k.append(t)

            b1f32 = bounce1[:].bitcast(F32)
            t1f32 = table1[:].bitcast(F32)
            b2f32 = bounce2[:].bitcast(F32)
            t2f32 = table2[:].bitcast(F32)

            # ---- Phase A ----
            for t in range(c.n_tiles):
                xts = []
                for k in range(K1):
                    xt = wk.tile([128, 128], F32, tag=f"xt{k}")
                    nc.sync.dma_start(out=xt[:], in_=t_xT[k * 128:(k + 1) * 128,
                                                         t * 128:(t + 1) * 128])
                    xts.append(xt)
                psf = ps.tile([128, hd1], F32, space="PSUM", tag="big")
                pse = pss.tile([128, 2 * h1], F32, space="PSUM", tag="small")
                for k in range(K1):
                    nc.tensor.matmul(out=psf[:], lhsT=xts[k][:], rhs=w1k[k][:, 0:hd1],
                                     start=(k == 0), stop=(k == K1 - 1))
                for k in range(K1):
                    nc.tensor.matmul(out=pse[:], lhsT=xts[k][:], rhs=w1k[k][:, hd1:NW1],
                                     start=(k == 0), stop=(k == K1 - 1))
                fbf = wk.tile([128, hd1], BF16, tag="fbf")
                nc.vector.tensor_copy(out=fbf[:], in_=psf[:])
                elr = wk.tile([128, 2 * h1], F32, tag="elr")
                nc.vector.tensor_copy(out=elr[:], in_=pse[:])
                r0, r1 = t * 128, (t + 1) * 128
                nc.sync.dma_start(out=bounce1[r0:r1, 0:hd1], in_=fbf[:])
                nc.sync.dma_start(out=b1f32[r0:r1, c.el1_f32:c.el1_f32 + h1],
                                  in_=elr[:, 0:h1])
                nc.sync.dma_start(out=erloc[r0:r1, 0:2 * h1], in_=elr[:])
            sent = wk.tile([1, 2 * h1], F32, tag="sent")
            nc.vector.memset(sent[:], SENT_NEG)
            nc.sync.dma_start(
                out=erloc[c.sent_row:c.sent_row + 1, 0:2 * h1], in_=sent[:])

            hrows = c.slice_pad // 2
            gh = c.n_cores * hrows
            for hh in range(2):
                r0, r1 = hh * hrows, (hh + 1) * hrows
                if fake_ag:
                    for cc in range(c.n_cores):
                        nc.sync.dma_start(
                            out=table1[hh * gh + cc * hrows:hh * gh + (cc + 1) * hrows, :],
                            in_=bounce1[r0:r1, :])
                else:
                    nc.gpsimd.collective_compute(
                        "AllGather", mybir.AluOpType.bypass,
                        replica_groups=[list(range(c.n_cores))],
                        ins=[bounce1[r0:r1, :]], outs=[table1[hh * gh:(hh + 1) * gh, :]])

            if c.slice_pad > c.slice:
                npad = c.slice_pad - c.slice
                nc.sync.dma_start(out=htab[c.slice:c.slice_pad, :],
                                  in_=zeros[:npad, :])

            # ---- Phase B: layer-1 edges ----
            for w in range(n_win):
                featg = gat.tile([128, J_W, c.row1_bf], BF16, tag="featg")
                _gather2(nc, featg, table1[:], srcidx, w * 128, W_E, c.row1_bf)
                erg = gat.tile([128, J_W, 64], F32, tag="erg")
                _gather2(nc, erg, erloc[:], dstidx, w * 128, W_E, 64)
                fg32 = featg[:].bitcast(F32)
                elv = fg32[:, :, c.el1_f32:c.el1_f32 + h1]
                erv = erg[:, :, h1:2 * h1]
                s = wk.tile([128, J_W, h1], F32, tag="s1")
                nc.vector.tensor_tensor(out=s[:], in0=elv, in1=erv,
                                        op=mybir.AluOpType.add)
                slr = wk.tile([128, J_W, h1], F32, tag="slr1")
                nc.vector.tensor_scalar_mul(slr[:], s[:], 0.2)
                slr2 = wk.tile([128, J_W, h1], F32, tag="slr1b")
                nc.vector.tensor_tensor(out=slr2[:], in0=slr[:], in1=s[:],
                                        op=mybir.AluOpType.max)
                pexps = wk.tile([128, J_W, h1], BF16, tag="pexps")
                nc.scalar.activation(out=pexps[:], in_=slr2[:],
                                     func=mybir.ActivationFunctionType.Exp)
                dre = gat.tile([128, J_W, 128], BF16, tag="dre")
                GRP = 4
                for g in range(0, J_W, GRP):
                    nc.scalar.activation(
                        out=dre[:, g:g + GRP],
                        in_=dstrel[:, w * J_W + g:w * J_W + g + GRP]
                            .to_broadcast([128, GRP, 128]),
                        func=mybir.ActivationFunctionType.Copy)

                psr = ps.tile([128, hd1], F32, space="PSUM", tag="big")
                psd = pss.tile([128, h1], F32, space="PSUM", tag="small")
                for ci in range(J_W):
                    ind = chk.tile([128, 128], BF16, tag="ind")
                    nc.vector.tensor_tensor(out=ind[:], in0=dre[:, ci],
                                            in1=iota[:], op=mybir.AluOpType.is_equal)
                    msg = chk.tile([128, d1, h1], BF16, tag="msg")
                    nc.vector.tensor_tensor(
                        out=msg[:],
                        in0=featg[:, ci, 0:hd1].rearrange("p (d h) -> p d h", h=h1),
                        in1=pexps[:, ci].unsqueeze(1).to_broadcast([128, d1, h1]),
                        op=mybir.AluOpType.mult)
                    nc.tensor.matmul(out=psr[:], lhsT=ind[:],
                                     rhs=msg[:].rearrange("p d h -> p (d h)"),
                                     start=(ci == 0), stop=(ci == J_W - 1))
                    nc.tensor.matmul(out=psd[:], lhsT=ind[:], rhs=pexps[:, ci],
                                     start=(ci == 0), stop=(ci == J_W - 1))
                dg = wk.tile([128, h1], F32, tag="dg1")
                nc.vector.tensor_scalar_max(dg[:], psd[:], 1e-30)
                rec = wk.tile([128, h1], F32, tag="rec1")
                nc.vector.reciprocal(out=rec[:], in_=dg[:])
                hwin = wk.tile([128, hd1], F32, tag="hwin")
                nc.vector.tensor_tensor(
                    out=hwin[:].rearrange("p (d h) -> p d h", h=h1),
                    in0=psr[:].rearrange("p (d h) -> p d h", h=h1),
                    in1=rec[:].unsqueeze(1).to_broadcast([128, d1, h1]),
                    op=mybir.AluOpType.mult)
                if has_b[0]:
                    nc.vector.tensor_tensor(out=hwin[:], in0=hwin[:], in1=b1t[:],
                                            op=mybir.AluOpType.add)
                if sim1:
                    nc.gpsimd.dma_start(
                        out=htab[(w % c.n_tiles) * 128:(w % c.n_tiles) * 128 + 128, :],
                        in_=hwin[:])
                else:
                    nc.gpsimd.indirect_dma_start(
                        out=htab[:],
                        out_offset=bass.IndirectOffsetOnAxis(ap=hidx[:, w:w + 1], axis=0),
                        in_=hwin[:], in_offset=None)

            # ---- Phase C: feat2 ----
            for t in range(c.n_tiles):
                ht = wk.tile([128, hd1], F32, tag="ht")
                nc.sync.dma_start(out=ht[:], in_=htab[t * 128:(t + 1) * 128, :])
                hTs = []
                for k in range(K2):
                    pst = ps.tile([128, 128], F32, space="PSUM", tag="big")
                    nc.tensor.transpose(out=pst[:], in_=ht[:, k * 128:(k + 1) * 128],
                                        identity=ident[:])
                    hT = wk.tile([128, 128], F32, tag=f"hT{k}")
                    nc.vector.tensor_copy(out=hT[:], in_=pst[:])
                    hTs.append(hT)
                psf2 = ps.tile([128, NW2], F32, space="PSUM", tag="big")
                for k in range(K2):
                    nc.tensor.matmul(out=psf2[:], lhsT=hTs[k][:], rhs=w2k[k][:],
                                     start=(k == 0), stop=(k == K2 - 1))
                f2bf = wk.tile([128, d2], BF16, tag="f2bf")
                nc.vector.tensor_copy(out=f2bf[:], in_=psf2[:, 0:d2])
                elr2 = wk.tile([128, 2], F32, tag="elr2")
                nc.vector.tensor_copy(out=elr2[:], in_=psf2[:, d2:d2 + 2])
                r0, r1 = t * 128, (t + 1) * 128
                nc.sync.dma_start(out=bounce2[r0:r1, 0:d2], in_=f2bf[:])
                nc.sync.dma_start(out=b2f32[r0:r1, c.el2_f32:c.el2_f32 + 1],
                                  in_=elr2[:, 0:1])
                nc.sync.dma_start(out=er2loc[r0:r1, 0:2], in_=elr2[:])
            sent2 = wk.tile([1, 2], F32, tag="sent2")
            nc.vector.memset(sent2[:], SENT_NEG)
            nc.sync.dma_start(
                out=er2loc[c.sent_row:c.sent_row + 1, 0:2], in_=sent2[:])

            for hh in range(2):
                r0, r1 = hh * hrows, (hh + 1) * hrows
                if fake_ag:
                    for cc in range(c.n_cores):
                        nc.sync.dma_start(
                            out=table2[hh * gh + cc * hrows:hh * gh + (cc + 1) * hrows, :],
                            in_=bounce2[r0:r1, :])
                else:
                    nc.gpsimd.collective_compute(
                        "AllGather", mybir.AluOpType.bypass,
                        replica_groups=[list(range(c.n_cores))],
                        ins=[bounce2[r0:r1, :]], outs=[table2[hh * gh:(hh + 1) * gh, :]])

            # ---- Phase D: layer-2 edges + pooling ----
            pspl = pspool.tile([128, d2], F32, space="PSUM", tag="pspl")
            for w in range(n_win):
                f2g = gat.tile([128, J_W, c.row2_bf], BF16, tag="f2g")
                _gather2(nc, f2g, table2[:], srcidx, w * 128, W_E, c.row2_bf)
                er2g = gat.tile([128, J_W, 64], F32, tag="er2g")
                _gather2(nc, er2g, er2loc[:], dstidx, w * 128, W_E, 64)
                f2g32 = f2g[:].bitcast(F32)
                el2v = f2g32[:, :, c.el2_f32:c.el2_f32 + 1]
                er2v = er2g[:, :, 1:2]
                s = wk.tile([128, J_W, 1], F32, tag="s2")
                nc.vector.tensor_tensor(out=s[:], in0=el2v, in1=er2v,
                                        op=mybir.AluOpType.add)
                slr = wk.tile([128, J_W, 1], F32, tag="slr2_")
                nc.vector.tensor_scalar_mul(slr[:], s[:], 0.2)
                slr2 = wk.tile([128, J_W, 1], F32, tag="slr2b")
                nc.vector.tensor_tensor(out=slr2[:], in0=slr[:], in1=s[:],
                                        op=mybir.AluOpType.max)
                pexp2 = gat.tile([128, J_W, 1, d2], BF16, tag="pexp2")
                dre = gat.tile([128, J_W, 128], BF16, tag="dre2")
                GRP = 4
                for g in range(0, J_W, GRP):
                    nc.scalar.activation(
                        out=dre[:, g:g + GRP],
                        in_=dstrel[:, w * J_W + g:w * J_W + g + GRP]
                            .to_broadcast([128, GRP, 128]),
                        func=mybir.ActivationFunctionType.Copy)
                    nc.scalar.activation(
                        out=pexp2[:, g:g + GRP],
                        in_=slr2[:, g:g + GRP].to_broadcast([128, GRP, 1, d2]),
                        func=mybir.ActivationFunctionType.Exp)

                psr2 = ps.tile([128, d2], F32, space="PSUM", tag="big")
                psd2 = pss.tile([128, 8], F32, space="PSUM", tag="small")
                for ci in range(J_W):
                    ind = chk.tile([128, 128], BF16, tag="ind2")
                    nc.vector.tensor_tensor(out=ind[:], in0=dre[:, ci],
                                            in1=iota[:], op=mybir.AluOpType.is_equal)
                    msg = chk.tile([128, d2], BF16, tag="msg2")
                    nc.vector.tensor_tensor(out=msg[:], in0=f2g[:, ci, 0:d2],
                                            in1=pexp2[:, ci, 0],
                                            op=mybir.AluOpType.mult)
                    nc.tensor.matmul(out=psr2[:], lhsT=ind[:], rhs=msg[:],
                                     start=(ci == 0), stop=(ci == J_W - 1))
                    nc.tensor.matmul(out=psd2[:, 0:1], lhsT=ind[:],
                                     rhs=pexp2[:, ci, :, 0],
                                     start=(ci == 0), stop=(ci == J_W - 1))
                dg = wk.tile([128, 1], F32, tag="dg2")
                nc.vector.tensor_scalar_max(dg[:], psd2[:, 0:1], 1e-30)
                rec = wk.tile([128, 1], F32, tag="rec2")
                nc.vector.reciprocal(out=rec[:], in_=dg[:])
                h2w = wk.tile([128, d2], F32, tag="h2w")
                nc.vector.tensor_tensor(out=h2w[:], in0=psr2[:],
                                        in1=rec[:].to_broadcast([128, d2]),
                                        op=mybir.AluOpType.mult)
                if has_b[1]:
                    nc.vector.tensor_tensor(out=h2w[:], in0=h2w[:], in1=b2t[:],
                                            op=mybir.AluOpType.add)
                gind = wk.tile([128, 128], F32, tag="gind")
                nc.vector.tensor_tensor(
                    out=gind[:],
                    in0=gidw[:, w:w + 1].to_broadcast([128, 128]),
                    in1=iota[:], op=mybir.AluOpType.is_equal)
                nc.tensor.matmul(out=pspl[:], lhsT=gind[:], rhs=h2w[:],
                                 start=(w == 0), stop=(w == n_win - 1))

            poolsb = res.tile([128, d2], F32)
            nc.vector.tensor_copy(out=poolsb[:], in_=pspl[:])
            nc.sync.dma_start(out=t_pool[:], in_=poolsb[:])

    nc.compile()
    return nc


def host_reduce(cfg, results, counts):
    pool_sum = np.zeros((128, cfg.d2), np.float32)
    for cc in range(cfg.n_cores):
        pool_sum += np.asarray(results[cc]["pool"], np.float32)
    hg = pool_sum[:cfg.n_graphs] / np.maximum(counts, 1.0)[:, None]
    return hg.reshape(cfg.n_graphs, 1, cfg.d2).astype(np.float32)




_PROG_CACHE = {}


def kernel(**inputs):
    """GAT forward on 8 trn2 NeuronCores. Full inputs in, [128,1,128] f32 out."""
    import concourse.bass_utils as bass_utils
    cfg = GatConfig()
    per_core, counts, n_win, has_b = build_host_data(cfg, **inputs)
    key = (n_win, has_b)
    nc = _PROG_CACHE.get(key)
    if nc is None:
        nc = build_program(cfg, n_win, has_b=has_b)
        _PROG_CACHE[key] = nc
    res = bass_utils.run_bass_kernel_spmd(nc, per_core,
                                          core_ids=list(range(cfg.n_cores)))
    return host_reduce(cfg, res.results, counts)

